# revision 1
# baseline (speedup 1.0000x reference)
"""Trainium2 Bass kernel for nn_GunnarODE: neural CDE with hermite spline control.

Contract: kernel(**inputs) takes FULL unsharded inputs (ts, us, ys, W1, b1,
W2, b2, batch_size) and returns the FULL (B, L, Y) output. Internally shards
the batch across 8 NeuronCores (pure data parallel), runs a Bass/Tile kernel
per core, and reassembles.

Algorithm notes (derived from the reference):
  - x = concat([t, us]) with unit-spaced knots (ts is arange) => dt == 1.
  - Hermite backward-difference spline derivative at substep s_i = i/4 of
    interval k reduces to dXdt_i = alpha_i * slope_{k-1} + beta_i * slope_k
    with alpha_i = 1-4s+3s^2, beta_i = 4s-3s^2 (alpha+beta=1), i.e. a linear
    blend of (u_{k-1}, u_k, u_{k+1}); the time channel has dXdt == 1.
  - Per Euler substep: h = tanh(z@W1.T+b1); vf = tanh(h@W2.T+b2) viewed as
    (Y=16, C=9); z += 0.25 * einsum(vf, dXdt).
  - On device everything is kept transposed (feature on partitions, batch on
    the free dim). The 144 vf rows are split into 128 "ctrl" rows
    (r=(c-1)*16+y for channels c=1..8) and 16 "time" rows (y*9).
  - All matmuls are fp32: the ODE amplifies per-step rounding ~1e5x, so
    reduced-precision matmuls (fp32r/bf16) fail the accuracy budget.
"""
import sys
if '/opt/trn_rl_repo' not in sys.path:
    sys.path.insert(0, '/opt/trn_rl_repo')

import numpy as np

N_CORES = 8
L = 512
B_TOT = 4096
U = 8
Y = 16
H = 128
C = U + 1
NI = L - 1            # intervals
HSTEP = 0.25          # dt / SUBSTEPS with dt == 1
B_LOC = B_TOT // N_CORES  # 512

ALPHA = [1.0, 0.1875, -0.25, -0.3125]
BETA = [0.0, 0.8125, 1.25, 1.3125]

_BUILD_CACHE = {}


def _host_constants(W1, b1, W2, b2):
    """Precompute transposed/permuted constant matrices (host-side, free)."""
    rowmap = np.array([(r % 16) * 9 + (r // 16 + 1) for r in range(128)])
    cst = {}
    cst["W1T"] = np.ascontiguousarray(W1.T)                        # (16,128)
    cst["W2aT"] = np.ascontiguousarray(W2[rowmap, :].T)            # (128,128)
    cst["W2bT"] = np.ascontiguousarray(W2[np.arange(16) * 9, :].T)  # (128,16)
    cst["b1c"] = np.ascontiguousarray(b1[:, None])                 # (128,1)
    cst["b2c"] = np.ascontiguousarray(b2[rowmap][:, None])         # (128,1)
    cst["b2t"] = np.ascontiguousarray(b2[np.arange(16) * 9][:, None])  # (16,1)
    abc = np.zeros((4, 24, 128), dtype=np.float32)
    for i in range(4):
        for r in range(128):
            c = r // 16 + 1
            abc[i, 0 * 8 + c - 1, r] = -ALPHA[i]
            abc[i, 1 * 8 + c - 1, r] = ALPHA[i] - BETA[i]
            abc[i, 2 * 8 + c - 1, r] = BETA[i]
    cst["Abc"] = abc                                               # (4,24,128)
    # hpre-state update matrices: hpre += (h*W1*Sel^T) @ tmp + (h*W1) @ vft
    w1selt = np.zeros((128, 128), dtype=np.float32)  # [r, j] = h*W1[j, r%16]
    for r in range(128):
        w1selt[r, :] = HSTEP * W1[:, r % 16]
    cst["W1SelT"] = w1selt
    cst["W1hT"] = (HSTEP * W1.T)                                   # (16,128)
    # output reconstruction: z = pinv(W1) @ hpre  (W1 is 128x16, cond ~2)
    R = np.linalg.pinv(W1.astype(np.float64)).astype(np.float32)   # (16,128)
    cst["RT"] = np.ascontiguousarray(R.T)                          # (128,16)
    return {k: v.astype(np.float32) for k, v in cst.items()}


def _build(n_intervals=NI):
    """Build + compile the Bass module (cached per interval count)."""
    key = n_intervals
    if key in _BUILD_CACHE:
        return _BUILD_CACHE[key]

    import concourse.bass as bass
    import concourse.bacc as bacc
    import concourse.tile as tile
    from concourse import mybir

    F32 = mybir.dt.float32
    TANH = mybir.ActivationFunctionType.Tanh
    MULT = mybir.AluOpType.mult
    ADD = mybir.AluOpType.add

    nc = bacc.Bacc("TRN2", target_bir_lowering=False, debug=False,
                   num_devices=N_CORES)

    d_us3 = nc.dram_tensor("us3", (n_intervals, 24, B_LOC), F32, kind="ExternalInput")
    d_ys0 = nc.dram_tensor("ys0T", (16, B_LOC), F32, kind="ExternalInput")
    d_W1T = nc.dram_tensor("W1T", (16, 128), F32, kind="ExternalInput")
    d_W2aT = nc.dram_tensor("W2aT", (128, 128), F32, kind="ExternalInput")
    d_W2bT = nc.dram_tensor("W2bT", (128, 16), F32, kind="ExternalInput")
    d_b1 = nc.dram_tensor("b1c", (128, 1), F32, kind="ExternalInput")
    d_b2c = nc.dram_tensor("b2c", (128, 1), F32, kind="ExternalInput")
    d_b2t = nc.dram_tensor("b2t", (16, 1), F32, kind="ExternalInput")
    d_Abc = nc.dram_tensor("Abc", (4, 24, 128), F32, kind="ExternalInput")
    d_W1SelT = nc.dram_tensor("W1SelT", (128, 128), F32, kind="ExternalInput")
    d_W1hT = nc.dram_tensor("W1hT", (16, 128), F32, kind="ExternalInput")
    d_RT = nc.dram_tensor("RT", (128, 16), F32, kind="ExternalInput")
    d_out = nc.dram_tensor("out", (n_intervals, 16, B_LOC), F32, kind="ExternalOutput")

    with tile.TileContext(nc) as tc:
        with (
            tc.tile_pool(name="consts", bufs=1) as consts,
            tc.tile_pool(name="zpool", bufs=3) as zpool,
            tc.tile_pool(name="work", bufs=2) as work,
            tc.tile_pool(name="u3p", bufs=3) as u3p,
            tc.tile_pool(name="ps1", bufs=1, space="PSUM") as ps1,
            tc.tile_pool(name="ps2", bufs=2, space="PSUM") as ps2,
        ):
            W1T = consts.tile([16, 128], F32)
            W2aT = consts.tile([128, 128], F32)
            W2bT = consts.tile([128, 16], F32)
            b1c = consts.tile([128, 1], F32)
            b2c = consts.tile([128, 1], F32)
            b2t = consts.tile([16, 1], F32)
            W1SelT = consts.tile([128, 128], F32)
            W1hT = consts.tile([16, 128], F32)
            RT = consts.tile([128, 16], F32)
            Abc = [consts.tile([24, 128], F32, name=f"Abc{i}") for i in range(4)]
            nc.sync.dma_start(W1T[:], d_W1T.ap())
            nc.sync.dma_start(W2aT[:], d_W2aT.ap())
            nc.sync.dma_start(W2bT[:], d_W2bT.ap())
            nc.sync.dma_start(b1c[:], d_b1.ap())
            nc.sync.dma_start(b2c[:], d_b2c.ap())
            nc.sync.dma_start(b2t[:], d_b2t.ap())
            nc.sync.dma_start(W1SelT[:], d_W1SelT.ap())
            nc.sync.dma_start(W1hT[:], d_W1hT.ap())
            nc.sync.dma_start(RT[:], d_RT.ap())
            for i in range(4):
                nc.sync.dma_start(Abc[i][:], d_Abc.ap()[i])

            z0 = zpool.tile([16, B_LOC], F32, tag="z")
            nc.sync.dma_start(z0[:], d_ys0.ap())

            # hpre is THE state: a persistent PSUM accumulator holding W1 @ z.
            # Each substep adds W1 @ dz via one K=128 + one K=16 matmul; z is
            # only reconstructed per interval for output via R = pinv(W1).
            hpre = ps1.tile([128, B_LOC], F32, tag="hpre")
            nc.tensor.matmul(hpre[:], W1T[:], z0[:], start=True, stop=False,
                             skip_group_check=True)

            HB = B_LOC // 2
            u3s = {}

            def load_u3(k):
                if k < n_intervals:
                    t = u3p.tile([24, B_LOC], F32, tag="u3", name=f"u3_{k}")
                    nc.sync.dma_start(t[:], d_us3.ap()[k])
                    u3s[k] = t

            load_u3(0)
            load_u3(1)
            for k in range(n_intervals):
                load_u3(k + 2)
                u3 = u3s.pop(k)
                for i in range(4):
                    dXb = ps2.tile([128, B_LOC], F32, tag="dXb")
                    # dXb halves fill the PE's tanh windows
                    nc.tensor.matmul(dXb[:, :HB], Abc[i][:], u3[:, :HB],
                                     start=True, stop=True)
                    th = work.tile([128, B_LOC], F32, tag="th")
                    # column-split pipeline: tanh_h half 0 -> MM2a half 0
                    # overlaps tanh_h half 1 -> MM2a half 1
                    nc.scalar.activation(th[:, :HB], hpre[:, :HB], TANH, bias=b1c[:])
                    nc.scalar.activation(th[:, HB:], hpre[:, HB:], TANH, bias=b1c[:])
                    vfc_h = [ps1.tile([128, HB], F32, tag=f"vfc{h}", name=f"vfc{h}_{k}_{i}")
                             for h in range(2)]
                    nc.tensor.matmul(vfc_h[0][:], W2aT[:], th[:, :HB],
                                     start=True, stop=True)
                    nc.tensor.matmul(vfc_h[1][:], W2aT[:], th[:, HB:],
                                     start=True, stop=True)
                    vft_ps = ps1.tile([16, B_LOC], F32, tag="vft")
                    nc.tensor.matmul(vft_ps[:], W2bT[:], th[:], start=True, stop=True)
                    nc.tensor.matmul(dXb[:, HB:], Abc[i][:], u3[:, HB:],
                                     start=True, stop=True)
                    vfc = work.tile([128, B_LOC], F32, tag="vfcs")
                    nc.scalar.activation(vfc[:, :HB], vfc_h[0][:], TANH, bias=b2c[:])
                    nc.scalar.activation(vfc[:, HB:], vfc_h[1][:], TANH, bias=b2c[:])
                    vft = work.tile([16, B_LOC], F32, tag="vfts")
                    nc.scalar.activation(vft[:], vft_ps[:], TANH, bias=b2t[:])
                    tmp = work.tile([128, B_LOC], F32, tag="tmp")
                    nc.vector.tensor_tensor(tmp[:, :HB], vfc[:, :HB], dXb[:, :HB],
                                            MULT)
                    # chain-critical state update, half-pipelined:
                    # hpre += (h*W1*Sel^T)@tmp + (h*W1)@vft
                    nc.tensor.matmul(hpre[:, :HB], W1SelT[:], tmp[:, :HB],
                                     start=False, stop=False, skip_group_check=True)
                    nc.vector.tensor_tensor(tmp[:, HB:], vfc[:, HB:], dXb[:, HB:],
                                            MULT)
                    nc.tensor.matmul(hpre[:, HB:], W1SelT[:], tmp[:, HB:],
                                     start=False, stop=False, skip_group_check=True)
                    nc.tensor.matmul(hpre[:], W1hT[:], vft[:], start=False,
                                     stop=False, skip_group_check=True)
                # per-interval output: z_{k+1} = pinv(W1) @ hpre
                hps = work.tile([128, B_LOC], F32, tag="hps")
                nc.vector.tensor_copy(hps[:], hpre[:])
                zt_ps = ps2.tile([16, B_LOC], F32, tag="ztp")
                nc.tensor.matmul(zt_ps[:], RT[:], hps[:], start=True, stop=True)
                zout = zpool.tile([16, B_LOC], F32, tag="z")
                nc.vector.tensor_copy(zout[:], zt_ps[:])
                nc.sync.dma_start(d_out.ap()[k], zout[:])

    nc.compile()
    _BUILD_CACHE[key] = nc
    return nc


def _prep_core_inputs(us, ys, cst, core, n_intervals):
    b0 = core * B_LOC
    usc = np.ascontiguousarray(us[:, b0:b0 + B_LOC, :].transpose(0, 2, 1))  # (L,8,B)
    us_ext = np.concatenate([2.0 * usc[:1] - usc[1:2], usc], axis=0)  # (L+1,8,B)
    sw = np.lib.stride_tricks.sliding_window_view(us_ext, 3, axis=0)  # (L-1,8,B,3)
    us3 = np.ascontiguousarray(sw.transpose(0, 3, 1, 2).reshape(L - 1, 24, B_LOC))
    us3 = us3[:n_intervals].astype(np.float32)
    ys0T = np.ascontiguousarray(ys[0, b0:b0 + B_LOC, :].T).astype(np.float32)
    m = {"us3": us3, "ys0T": ys0T}
    m.update({k: v for k, v in cst.items() if k not in ("Abc",)})
    m["Abc"] = cst["Abc"]
    return m


def kernel(ts, us, ys, W1, b1, W2, b2, batch_size=None, n_intervals=NI):
    from concourse.bass_utils import run_bass_kernel_spmd

    us = np.asarray(us, dtype=np.float32)
    ys = np.asarray(ys, dtype=np.float32)
    cst = _host_constants(np.asarray(W1, np.float32), np.asarray(b1, np.float32),
                          np.asarray(W2, np.float32), np.asarray(b2, np.float32))
    nc = _build(n_intervals)
    in_maps = [_prep_core_inputs(us, ys, cst, c, n_intervals) for c in range(N_CORES)]
    res = run_bass_kernel_spmd(nc, in_maps, core_ids=list(range(N_CORES)))
    out = np.empty((B_TOT, n_intervals + 1, Y), dtype=np.float32)
    out[:, 0, :] = ys[0]
    for c in range(N_CORES):
        b0 = c * B_LOC
        out[b0:b0 + B_LOC, 1:, :] = res.results[c]["out"].transpose(2, 0, 1)
    kernel._last_results = res
    return out



# revision 3
# speedup vs baseline: 1.0604x; 1.0604x over previous
"""Trainium2 Bass kernel for nn_GunnarODE: neural CDE with hermite spline control.

Contract: kernel(**inputs) takes FULL unsharded inputs (ts, us, ys, W1, b1,
W2, b2, batch_size) and returns the FULL (B, L, Y) output. Internally shards
the batch across 8 NeuronCores (pure data parallel), runs a Bass/Tile kernel
per core, and reassembles.

Algorithm notes (derived from the reference):
  - x = concat([t, us]) with unit-spaced knots (ts is arange) => dt == 1.
  - Hermite backward-difference spline derivative at substep s_i = i/4 of
    interval k reduces to dXdt_i = alpha_i * slope_{k-1} + beta_i * slope_k
    with alpha_i = 1-4s+3s^2, beta_i = 4s-3s^2 (alpha+beta=1), i.e. a linear
    blend of (u_{k-1}, u_k, u_{k+1}); the time channel has dXdt == 1.
  - Per Euler substep: h = tanh(z@W1.T+b1); vf = tanh(h@W2.T+b2) viewed as
    (Y=16, C=9); z += 0.25 * einsum(vf, dXdt).
  - On device everything is kept transposed (feature on partitions, batch on
    the free dim). The 144 vf rows are split into 128 "ctrl" rows
    (r=(c-1)*16+y for channels c=1..8) and 16 "time" rows (y*9).
  - hpre = W1 @ z (+0 bias) is THE state, held in a persistent fp32 PSUM
    accumulator; per substep it is incremented by two accumulating matmuls,
    and z is reconstructed per interval via R = pinv(W1) for output only.

Precision schedule (validated by CPU error-amplification study):
  - The ODE is chaotic: a relative perturbation injected at interval k is
    amplified by G(k) at the end (G(0)~2e4, G(128)~48, G(320)~4). Graded
    precision: full fp32 matmuls for k < K0 = 128, fp16 matmuls (fp32 PSUM
    accumulate) for k >= K0. Simulated end-to-end rel-L2 vs the fp32
    reference ~ 8.8e-3 (budget 2e-2).
  - fp32 matmuls cost ~2.4 cyc/col + serial self-loading weights; fp16 are
    1 cyc/col with pipelined LDWEIGHTS, so the schedule also picks different
    loop bodies: full-width (fewer weight loads) for the PE-bound fp32
    region, half-split dual streams (ScalarE-bound) for the fp16 region.
"""
import sys
if '/opt/trn_rl_repo' not in sys.path:
    sys.path.insert(0, '/opt/trn_rl_repo')

import numpy as np

N_CORES = 8
L = 512
B_TOT = 4096
U = 8
Y = 16
H = 128
C = U + 1
NI = L - 1            # intervals
HSTEP = 0.25          # dt / SUBSTEPS with dt == 1
B_LOC = B_TOT // N_CORES  # 512
K0 = 128              # first fp16 interval

ALPHA = [1.0, 0.1875, -0.25, -0.3125]
BETA = [0.0, 0.8125, 1.25, 1.3125]

_BUILD_CACHE = {}


def _host_constants(W1, b1, W2, b2):
    """Precompute transposed/permuted constant matrices (host-side, free)."""
    rowmap = np.array([(r % 16) * 9 + (r // 16 + 1) for r in range(128)])
    cst = {}
    cst["W1T"] = np.ascontiguousarray(W1.T)                        # (16,128)
    cst["W2aT"] = np.ascontiguousarray(W2[rowmap, :].T)            # (128,128)
    cst["W2bT"] = np.ascontiguousarray(W2[np.arange(16) * 9, :].T)  # (128,16)
    cst["b1c"] = np.ascontiguousarray(b1[:, None])                 # (128,1)
    cst["b2c"] = np.ascontiguousarray(b2[rowmap][:, None])         # (128,1)
    cst["b2t"] = np.ascontiguousarray(b2[np.arange(16) * 9][:, None])  # (16,1)
    abc = np.zeros((4, 24, 128), dtype=np.float32)
    for i in range(4):
        for r in range(128):
            c = r // 16 + 1
            abc[i, 0 * 8 + c - 1, r] = -ALPHA[i]
            abc[i, 1 * 8 + c - 1, r] = ALPHA[i] - BETA[i]
            abc[i, 2 * 8 + c - 1, r] = BETA[i]
    cst["Abc"] = abc                                               # (4,24,128)
    # hpre-state update matrices: hpre += (h*W1*Sel^T) @ tmp + (h*W1) @ vft
    w1selt = np.zeros((128, 128), dtype=np.float32)  # [r, j] = h*W1[j, r%16]
    for r in range(128):
        w1selt[r, :] = HSTEP * W1[:, r % 16]
    cst["W1SelT"] = w1selt
    cst["W1hT"] = (HSTEP * W1.T)                                   # (16,128)
    # output reconstruction: z = pinv(W1) @ hpre  (W1 is 128x16, cond ~2)
    R = np.linalg.pinv(W1.astype(np.float64)).astype(np.float32)   # (16,128)
    cst["RT"] = np.ascontiguousarray(R.T)                          # (128,16)
    return {k: v.astype(np.float32) for k, v in cst.items()}


def _build(n_intervals=NI, k0=K0):
    """Build + compile the Bass module (cached per interval count)."""
    key = (n_intervals, k0)
    if key in _BUILD_CACHE:
        return _BUILD_CACHE[key]

    import concourse.bass as bass
    import concourse.bacc as bacc
    import concourse.tile as tile
    from concourse import mybir

    F32 = mybir.dt.float32
    F16 = mybir.dt.float16
    TANH = mybir.ActivationFunctionType.Tanh
    MULT = mybir.AluOpType.mult

    n32 = min(k0, n_intervals)       # intervals with fp32 body
    n16 = n_intervals - n32          # intervals with fp16 body

    nc = bacc.Bacc("TRN2", target_bir_lowering=False, debug=False,
                   num_devices=N_CORES)

    # fp32-region spline windows (n32,24,B) fp32; fp16-region ones in fp16
    if n32:
        d_us3a = nc.dram_tensor("us3a", (n32, 24, B_LOC), F32, kind="ExternalInput")
    if n16:
        d_us3b = nc.dram_tensor("us3b", (n16, 24, B_LOC), F16, kind="ExternalInput")
    d_ys0 = nc.dram_tensor("ys0T", (16, B_LOC), F32, kind="ExternalInput")
    d_W1T = nc.dram_tensor("W1T", (16, 128), F32, kind="ExternalInput")
    d_W2aT = nc.dram_tensor("W2aT", (128, 128), F32, kind="ExternalInput")
    d_W2bT = nc.dram_tensor("W2bT", (128, 16), F32, kind="ExternalInput")
    d_b1 = nc.dram_tensor("b1c", (128, 1), F32, kind="ExternalInput")
    d_b2c = nc.dram_tensor("b2c", (128, 1), F32, kind="ExternalInput")
    d_b2t = nc.dram_tensor("b2t", (16, 1), F32, kind="ExternalInput")
    d_Abc = nc.dram_tensor("Abc", (4, 24, 128), F32, kind="ExternalInput")
    d_W1SelT = nc.dram_tensor("W1SelT", (128, 128), F32, kind="ExternalInput")
    d_W1hT = nc.dram_tensor("W1hT", (16, 128), F32, kind="ExternalInput")
    d_RT = nc.dram_tensor("RT", (128, 16), F32, kind="ExternalInput")
    # fp16 copies of the constants for the late region
    d_W2aTh = nc.dram_tensor("W2aTh", (128, 128), F16, kind="ExternalInput")
    d_W2bTh = nc.dram_tensor("W2bTh", (128, 16), F16, kind="ExternalInput")
    d_Abch = nc.dram_tensor("Abch", (4, 24, 128), F16, kind="ExternalInput")
    d_W1SelTh = nc.dram_tensor("W1SelTh", (128, 128), F16, kind="ExternalInput")
    d_W1hTh = nc.dram_tensor("W1hTh", (16, 128), F16, kind="ExternalInput")
    d_out = nc.dram_tensor("out", (n_intervals, 16, B_LOC), F32, kind="ExternalOutput")

    with tile.TileContext(nc) as tc:
        with (
            tc.tile_pool(name="consts", bufs=1) as consts,
            tc.tile_pool(name="zpool", bufs=3) as zpool,
            tc.tile_pool(name="work", bufs=2) as work,
            tc.tile_pool(name="u3p", bufs=3) as u3p,
            tc.tile_pool(name="ps1", bufs=1, space="PSUM") as ps1,
            tc.tile_pool(name="ps2", bufs=2, space="PSUM") as ps2,
        ):
            W1T = consts.tile([16, 128], F32)
            W2aT = consts.tile([128, 128], F32)
            W2bT = consts.tile([128, 16], F32)
            b1c = consts.tile([128, 1], F32)
            b2c = consts.tile([128, 1], F32)
            b2t = consts.tile([16, 1], F32)
            W1SelT = consts.tile([128, 128], F32)
            W1hT = consts.tile([16, 128], F32)
            RT = consts.tile([128, 16], F32)
            Abc = [consts.tile([24, 128], F32, name=f"Abc{i}") for i in range(4)]
            W2aTh = consts.tile([128, 128], F16)
            W2bTh = consts.tile([128, 16], F16)
            W1SelTh = consts.tile([128, 128], F16)
            W1hTh = consts.tile([16, 128], F16)
            Abch = [consts.tile([24, 128], F16, name=f"Abch{i}") for i in range(4)]
            nc.sync.dma_start(W1T[:], d_W1T.ap())
            nc.sync.dma_start(W2aT[:], d_W2aT.ap())
            nc.sync.dma_start(W2bT[:], d_W2bT.ap())
            nc.sync.dma_start(b1c[:], d_b1.ap())
            nc.sync.dma_start(b2c[:], d_b2c.ap())
            nc.sync.dma_start(b2t[:], d_b2t.ap())
            nc.sync.dma_start(W1SelT[:], d_W1SelT.ap())
            nc.sync.dma_start(W1hT[:], d_W1hT.ap())
            nc.sync.dma_start(RT[:], d_RT.ap())
            nc.sync.dma_start(W2aTh[:], d_W2aTh.ap())
            nc.sync.dma_start(W2bTh[:], d_W2bTh.ap())
            nc.sync.dma_start(W1SelTh[:], d_W1SelTh.ap())
            nc.sync.dma_start(W1hTh[:], d_W1hTh.ap())
            for i in range(4):
                nc.sync.dma_start(Abc[i][:], d_Abc.ap()[i])
                nc.sync.dma_start(Abch[i][:], d_Abch.ap()[i])

            z0 = zpool.tile([16, B_LOC], F32, tag="z")
            nc.sync.dma_start(z0[:], d_ys0.ap())

            # hpre is THE state: a persistent PSUM accumulator holding W1 @ z.
            hpre = ps1.tile([128, B_LOC], F32, tag="hpre")
            nc.tensor.matmul(hpre[:], W1T[:], z0[:], start=True, stop=False,
                             skip_group_check=True)

            HB = B_LOC // 2
            u3s = {}

            def load_u3(k):
                if k < n_intervals:
                    if k < n32:
                        t = u3p.tile([24, B_LOC], F32, tag="u3", name=f"u3_{k}")
                        nc.sync.dma_start(t[:], d_us3a.ap()[k])
                    else:
                        t = u3p.tile([24, B_LOC], F16, tag="u3h", name=f"u3_{k}")
                        nc.sync.dma_start(t[:], d_us3b.ap()[k - n32])
                    u3s[k] = t

            def out_interval(k):
                # per-interval output: z_{k+1} = pinv(W1) @ hpre
                hps = work.tile([128, B_LOC], F32, tag="hps")
                nc.vector.tensor_copy(hps[:], hpre[:])
                zt_ps = ps2.tile([16, B_LOC], F32, tag="ztp")
                nc.tensor.matmul(zt_ps[:], RT[:], hps[:], start=True, stop=True)
                zout = zpool.tile([16, B_LOC], F32, tag="z")
                nc.vector.tensor_copy(zout[:], zt_ps[:])
                nc.sync.dma_start(d_out.ap()[k], zout[:])

            load_u3(0)
            load_u3(1)

            # ---------------- fp32 region: full-width body ----------------
            for k in range(n32):
                load_u3(k + 2)
                u3 = u3s.pop(k)
                for i in range(4):
                    dXb = ps2.tile([128, B_LOC], F32, tag="dXb")
                    nc.tensor.matmul(dXb[:], Abc[i][:], u3[:], start=True,
                                     stop=True)
                    th = work.tile([128, B_LOC], F32, tag="th")
                    nc.scalar.activation(th[:], hpre[:], TANH, bias=b1c[:])
                    vfc_ps = ps1.tile([128, B_LOC], F32, tag="vfcp",
                                      name=f"vfcp_{k}_{i}")
                    nc.tensor.matmul(vfc_ps[:], W2aT[:], th[:], start=True,
                                     stop=True)
                    vft_ps = ps1.tile([16, B_LOC], F32, tag="vft")
                    nc.tensor.matmul(vft_ps[:], W2bT[:], th[:], start=True,
                                     stop=True)
                    vfc = work.tile([128, B_LOC], F32, tag="vfcs")
                    nc.scalar.activation(vfc[:], vfc_ps[:], TANH, bias=b2c[:])
                    vft = work.tile([16, B_LOC], F32, tag="vfts")
                    nc.scalar.activation(vft[:], vft_ps[:], TANH, bias=b2t[:])
                    tmp = work.tile([128, B_LOC], F32, tag="tmp")
                    nc.vector.tensor_tensor(tmp[:], vfc[:], dXb[:], MULT)
                    nc.tensor.matmul(hpre[:], W1SelT[:], tmp[:], start=False,
                                     stop=False, skip_group_check=True)
                    nc.tensor.matmul(hpre[:], W1hT[:], vft[:], start=False,
                                     stop=False, skip_group_check=True)
                out_interval(k)

            # ------------- fp16 region: dual half-batch streams -------------
            for k in range(n32, n_intervals):
                load_u3(k + 2)
                u3 = u3s.pop(k)
                for i in range(4):
                    dXb = ps2.tile([128, B_LOC], F32, tag="dXb",
                                   name=f"dXb_{k}_{i}")
                    vfc_ps = ps1.tile([128, B_LOC], F32, tag="vfcp",
                                      name=f"vfcp_{k}_{i}")
                    vft_ps = ps1.tile([16, B_LOC], F32, tag="vft",
                                      name=f"vftp_{k}_{i}")
                    for h0, h1, s in ((0, HB, 0), (HB, B_LOC, 1)):
                        nc.tensor.matmul(dXb[:, h0:h1], Abch[i][:],
                                         u3[:, h0:h1], start=True, stop=True)
                        th = work.tile([128, HB], F16, tag=f"th{s}")
                        nc.scalar.activation(th[:], hpre[:, h0:h1], TANH,
                                             bias=b1c[:])
                        nc.tensor.matmul(vfc_ps[:, h0:h1], W2aTh[:], th[:],
                                         start=True, stop=True)
                        nc.tensor.matmul(vft_ps[:, h0:h1], W2bTh[:], th[:],
                                         start=True, stop=True)
                        vfc = work.tile([128, HB], F32, tag=f"vfcs{s}")
                        nc.scalar.activation(vfc[:], vfc_ps[:, h0:h1], TANH,
                                             bias=b2c[:])
                        vft = work.tile([16, HB], F16, tag=f"vfts{s}")
                        nc.scalar.activation(vft[:], vft_ps[:, h0:h1], TANH,
                                             bias=b2t[:])
                        tmp = work.tile([128, HB], F16, tag=f"tmp{s}")
                        nc.vector.tensor_tensor(tmp[:], vfc[:], dXb[:, h0:h1],
                                                MULT)
                        nc.tensor.matmul(hpre[:, h0:h1], W1SelTh[:], tmp[:],
                                         start=False, stop=False,
                                         skip_group_check=True)
                        nc.tensor.matmul(hpre[:, h0:h1], W1hTh[:], vft[:],
                                         start=False, stop=False,
                                         skip_group_check=True)
                out_interval(k)

    nc.compile()
    _BUILD_CACHE[key] = nc
    return nc


def _prep_core_inputs(us, ys, cst, core, n_intervals, k0=K0):
    b0 = core * B_LOC
    usc = np.ascontiguousarray(us[:, b0:b0 + B_LOC, :].transpose(0, 2, 1))  # (L,8,B)
    us_ext = np.concatenate([2.0 * usc[:1] - usc[1:2], usc], axis=0)  # (L+1,8,B)
    sw = np.lib.stride_tricks.sliding_window_view(us_ext, 3, axis=0)  # (L-1,8,B,3)
    us3 = np.ascontiguousarray(sw.transpose(0, 3, 1, 2).reshape(L - 1, 24, B_LOC))
    us3 = us3[:n_intervals]
    n32 = min(k0, n_intervals)
    ys0T = np.ascontiguousarray(ys[0, b0:b0 + B_LOC, :].T).astype(np.float32)
    m = {"ys0T": ys0T}
    if n32:
        m["us3a"] = us3[:n32].astype(np.float32)
    if n_intervals > n32:
        m["us3b"] = us3[n32:].astype(np.float16)
    m.update({k: v for k, v in cst.items() if k not in ("Abc",)})
    m["Abc"] = cst["Abc"]
    for k in ("W2aT", "W2bT", "W1SelT", "W1hT", "Abc"):
        m[k + "h" if k != "Abc" else "Abch"] = cst[k].astype(np.float16)
    return m


def kernel(ts, us, ys, W1, b1, W2, b2, batch_size=None, n_intervals=NI):
    from concourse.bass_utils import run_bass_kernel_spmd

    us = np.asarray(us, dtype=np.float32)
    ys = np.asarray(ys, dtype=np.float32)
    cst = _host_constants(np.asarray(W1, np.float32), np.asarray(b1, np.float32),
                          np.asarray(W2, np.float32), np.asarray(b2, np.float32))
    nc = _build(n_intervals)
    in_maps = [_prep_core_inputs(us, ys, cst, c, n_intervals) for c in range(N_CORES)]
    res = run_bass_kernel_spmd(nc, in_maps, core_ids=list(range(N_CORES)))
    out = np.empty((B_TOT, n_intervals + 1, Y), dtype=np.float32)
    out[:, 0, :] = ys[0]
    for c in range(N_CORES):
        b0 = c * B_LOC
        out[b0:b0 + B_LOC, 1:, :] = res.results[c]["out"].transpose(2, 0, 1)
    kernel._last_results = res
    return out
